# revision 1
# baseline (speedup 1.0000x reference)
"""Trainium2 Bass kernel for a binarized 4-layer MLP (eval mode).

Reference computation (per row of x [B=16384, 784]):
  h1 = x @ sign(w1).T + b1;  s1 = sign(bn1(h1))        (clip doesn't change sign)
  h2 = s1 @ sign(w2).T + b2; s2 = sign(bn2(h2))
  h3 = s2 @ sign(w3).T + b3; y3 = clip(bn3(h3), -1, 1)
  z  = y3 @ w4.T + b4;       out = log_softmax(z)

Sharding: pure data-parallel over the batch across 8 NeuronCores
(weights replicated, no collectives).

Numerics:
  - L1: x is split exactly into two fp16 terms (a = fp16(x), b = fp16(x - a),
    residual <= 2^-22 |x|); the +-1 weights are exact in fp16, so two fp16
    matmuls accumulated in fp32 PSUM give fp32-class accuracy at 2-byte rate.
  - L2/L3: both operands are exactly +-1/0 in fp8e4 -> DoubleRow fp8 matmuls
    (2 k-tiles per pass) produce bit-exact integer sums in fp32 PSUM.
  - BN + bias folding: bn(h + b) = A*h + C with A = g*rsqrt(v+eps),
    C = A*(b - m) + beta, applied per-partition by the Sign/Identity
    activations (fp32 internally).
"""

import sys

if "/opt/trn_rl_repo" not in sys.path:
    sys.path.insert(0, "/opt/trn_rl_repo")

import numpy as np

D_IN, H1, H2, H3, NCLS = 784, 3072, 1536, 768, 10
B, NCORES = 16384, 8
BC = B // NCORES          # batch rows per core
NB = 256                  # batch columns processed per chunk
KP = 112                  # L1 k-tile partition size (784 = 7 * 112)
K1T = D_IN // KP          # 7
M1, M2, M3 = H1 // 128, H2 // 128, H3 // 128   # 24, 12, 6
K2P, K3P = H1 // 256, H2 // 256                # DoubleRow k-pair iters: 12, 6
K4T = H3 // 128                                # 6
BN_EPS = 1e-5

_cached = {}


def _build(bc):
    import concourse.bacc as bacc
    import concourse.mybir as mybir
    import concourse.tile as tile

    dt = mybir.dt
    AF = mybir.ActivationFunctionType
    PM = mybir.MatmulPerfMode
    ALU = mybir.AluOpType

    assert bc % NB == 0 and NB % 128 == 0
    gbts = bc // 128  # output row-tiles per core

    nc = bacc.Bacc("TRN2", target_bir_lowering=False, debug=False,
                   num_devices=NCORES)

    xt = nc.declare_dram_parameter("xt", [D_IN, bc], dt.float32, isOutput=False)
    w1t = nc.declare_dram_parameter("w1t", [D_IN, H1], dt.float16, isOutput=False)
    w2t = nc.declare_dram_parameter("w2t", [H1, H2], dt.float8e4, isOutput=False)
    w3t = nc.declare_dram_parameter("w3t", [H2, H3], dt.float8e4, isOutput=False)
    w4t = nc.declare_dram_parameter("w4t", [H3, NCLS], dt.bfloat16, isOutput=False)
    a1s = nc.declare_dram_parameter("a1s", [128, M1], dt.float32, isOutput=False)
    c1s = nc.declare_dram_parameter("c1s", [128, M1], dt.float32, isOutput=False)
    a2s = nc.declare_dram_parameter("a2s", [128, M2], dt.float32, isOutput=False)
    c2s = nc.declare_dram_parameter("c2s", [128, M2], dt.float32, isOutput=False)
    a3s = nc.declare_dram_parameter("a3s", [128, M3], dt.float32, isOutput=False)
    c3s = nc.declare_dram_parameter("c3s", [128, M3], dt.float32, isOutput=False)
    b4s = nc.declare_dram_parameter("b4s", [128, NCLS], dt.float32, isOutput=False)
    out = nc.declare_dram_parameter("out", [bc, NCLS], dt.float32, isOutput=True)

    with tile.TileContext(nc) as tc, \
            tc.tile_pool(name="wts", bufs=1) as wp, \
            tc.tile_pool(name="xin", bufs=2) as xp, \
            tc.tile_pool(name="act", bufs=2) as ap_, \
            tc.tile_pool(name="eps", bufs=2) as ep, \
            tc.tile_pool(name="ps", bufs=4, space="PSUM") as ps, \
            tc.tile_pool(name="ps4", bufs=2, space="PSUM") as ps4:

        # ---- startup-critical transfers first: consts, chunk-0 x, then w1.
        # w2/w3 streams are dependency-chained onto chunk-0 compute
        # milestones below so they don't steal HBM bandwidth at startup.
        a1sb = wp.tile([128, M1], dt.float32, tag="a1")
        c1sb = wp.tile([128, M1], dt.float32, tag="c1")
        a2sb = wp.tile([128, M2], dt.float32, tag="a2")
        c2sb = wp.tile([128, M2], dt.float32, tag="c2")
        a3sb = wp.tile([128, M3], dt.float32, tag="a3")
        c3sb = wp.tile([128, M3], dt.float32, tag="c3")
        b4sb = wp.tile([128, NCLS], dt.float32, tag="b4")
        for sb, drh in ((a1sb, a1s), (c1sb, c1s), (a2sb, a2s), (c2sb, c2s),
                        (a3sb, a3s), (c3sb, c3s), (b4sb, b4s)):
            nc.sync.dma_start(sb[:], drh[:])

        # L1 runs on wide batch groups (W columns) to halve matmul count;
        # L2-L4 iterate over NB-column halves of each group.
        W = 2 * NB if bc % (2 * NB) == 0 else NB
        ngroups = bc // W
        halves = W // NB

        def load_x(g):
            # returns per-k-tile AP lists for the two fp16 streams
            cs = slice(g * W, (g + 1) * W)
            xap = xt.ap()[:, cs]
            if g == 0:
                # group 0 is startup-latency critical: separate tiles per
                # k-tile so each matmul depends only on its own k-tile's
                # DMA+split (tile-granular deps would otherwise serialize
                # the first matmul behind the whole load)
                xas, xbs = [], []
                xdma = None
                for k in range(K1T):
                    xik = xp.tile([KP, W], dt.float32, tag=f"xi{k}", bufs=1,
                                  name=f"xi{k}")
                    d = nc.sync.dma_start(xik[:], xap[k * KP:(k + 1) * KP, :])
                    xdma = xdma or d
                    xak = xp.tile([KP, W], dt.float16, tag=f"xa{k}", bufs=1,
                                  name=f"xa{k}")
                    xbk = xp.tile([KP, W], dt.float16, tag=f"xb{k}", bufs=1,
                                  name=f"xb{k}")
                    nc.vector.tensor_copy(xak[:], xik[:])
                    nc.vector.tensor_sub(xbk[:], xik[:], xak[:])
                    xas.append(xak)
                    xbs.append(xbk)
                return xas, xbs, xdma
            xin = xp.tile([KP, K1T, W], dt.float32, tag="xin", bufs=1)
            xdma = nc.sync.dma_start(
                xin[:], xap.rearrange("(kt p) b -> p kt b", p=KP))
            xa = xp.tile([KP, K1T, W], dt.float16, tag="xa")
            xb = xp.tile([KP, K1T, W], dt.float16, tag="xb")
            for k0, k1 in ((0, 4), (4, K1T)):
                nc.vector.tensor_copy(xa[:, k0:k1, :], xin[:, k0:k1, :])
                nc.vector.tensor_sub(xb[:, k0:k1, :], xin[:, k0:k1, :],
                                     xa[:, k0:k1, :])
            return ([xa[:, k, :] for k in range(K1T)],
                    [xb[:, k, :] for k in range(K1T)], xdma)

        x0 = load_x(0)

        # (fp8-shipping w1 with an on-device upcast was tried and reverted:
        # fp8->fp16 casts run ~5x below line rate on both DVE and GpSimd.)
        # Per-k-tile tiles so matmuls depend only on their own w1 transfer.
        w1k = []
        for kt in range(K1T):
            wk = wp.tile([KP, H1], dt.float16, tag=f"w1_{kt}", name=f"w1_{kt}")
            nc.sync.dma_start(wk[:], w1t[kt * KP:(kt + 1) * KP, :])
            w1k.append(wk)

        w2sb = wp.tile([128, 2 * K2P, H2], dt.float8e4, tag="w2")
        w2_dmas = [
            nc.sync.dma_start(w2sb[:, kt, :], w2t[kt * 128:(kt + 1) * 128, :])
            for kt in range(2 * K2P)
        ]
        w3sb = wp.tile([128, 2 * K3P, H3], dt.float8e4, tag="w3")
        w3_dmas = [
            nc.sync.dma_start(w3sb[:, kt, :], w3t[kt * 128:(kt + 1) * 128, :])
            for kt in range(2 * K3P)
        ]
        w4sb = wp.tile([128, K4T, NCLS], dt.bfloat16, tag="w4")
        nc.sync.dma_start(w4sb[:], w4t.ap().rearrange("(kt p) n -> p kt n", p=128))

        zout = wp.tile([128, gbts, NCLS], dt.float32, tag="zout")
        ssum = wp.tile([128, gbts], dt.float32, tag="ssum")
        lsum = wp.tile([128, gbts], dt.float32, tag="lsum")

        def emit_epilogue(lo, hi):
            # log_softmax over the free dim; |z| is small so no max-shift
            for g in range(lo, hi):
                e = ep.tile([128, NCLS], dt.float32, tag="e")
                nc.scalar.activation(e[:], zout[:, g, :], AF.Exp,
                                     accum_out=ssum[:, g:g + 1])
            nc.scalar.activation(lsum[:, lo:hi], ssum[:, lo:hi], AF.Ln)
            for g in range(lo, hi):
                nc.vector.tensor_scalar(zout[:, g, :], zout[:, g, :],
                                        lsum[:, g:g + 1], None,
                                        op0=ALU.subtract)
            nc.sync.dma_start(
                out.ap()[lo * 128:hi * 128, :].rearrange("(g p) n -> p g n",
                                                         p=128),
                zout[:, lo:hi, :])

        prev_act0 = None
        for g in range(ngroups):
            if g == 0:
                xa, xb, _ = x0
            else:
                xa, xb, xdma = load_x(g)
                if prev_act0 is not None:
                    # keep ~one group of x lookahead; don't fight the
                    # startup transfers
                    tile.add_dep_helper(xdma.ins, prev_act0.ins, sync=True,
                                        reason="x prefetch staging")

            # ---- L1: [784 -> 3072], two fp16 streams into fp32 PSUM
            h1sb = ap_.tile([128, 2 * K2P, W], dt.float8e4, tag="h1")

            def l1_sign(mt, pt):
                act = nc.scalar.activation(h1sb[:, mt, :], pt[:], AF.Sign,
                                           bias=c1sb[:, mt:mt + 1],
                                           scale=a1sb[:, mt:mt + 1])
                if g == 0:
                    # stage w2/w3 weight streams behind group-0 L1 progress
                    # so they don't starve the startup transfers
                    for wd_list, base in ((w2_dmas, 0), (w3_dmas, M1 // 2)):
                        for kt2, wd in enumerate(wd_list):
                            if base + kt2 // 2 == mt:
                                tile.add_dep_helper(
                                    wd.ins, act.ins, sync=True,
                                    reason="weight stream staging")
                if mt == 0:
                    return act
                return None

            # (kt-outer orderings cycling >4 PSUM banks HAM-oscillate the PE
            # and slow every matmul class 5-20% -- keep total live banks <=6.)
            if g == 0:
                # kt-outer over psum-groups of 4: the PE consumes each w1/x
                # k-tile as its DMA lands instead of idling until the whole
                # w1 stream arrives; the HAM-cold ramp hides in that window.
                for mg in range(0, M1, 4):
                    pts = [ps.tile([128, W], dt.float32, tag="ps",
                                   name=f"pt{i}") for i in range(4)]
                    for kt in range(K1T):
                        for i in range(4):
                            mt = mg + i
                            lhs = w1k[kt][:, mt * 128:(mt + 1) * 128]
                            nc.tensor.matmul(pts[i][:], lhs, xa[kt][:],
                                             start=(kt == 0), stop=False)
                            nc.tensor.matmul(pts[i][:], lhs, xb[kt][:],
                                             start=False,
                                             stop=(kt == K1T - 1))
                    for i in range(4):
                        a = l1_sign(mg + i, pts[i])
                        prev_act0 = a or prev_act0
            else:
                for mt in range(M1):
                    pt = ps.tile([128, W], dt.float32, tag="ps")
                    for kt in range(K1T):
                        lhs = w1k[kt][:, mt * 128:(mt + 1) * 128]
                        nc.tensor.matmul(pt[:], lhs, xa[kt][:],
                                         start=(kt == 0), stop=False)
                        nc.tensor.matmul(pt[:], lhs, xb[kt][:],
                                         start=False, stop=(kt == K1T - 1))
                    a = l1_sign(mt, pt)
                    prev_act0 = a or prev_act0

            for h in range(halves):
                hs = slice(h * NB, (h + 1) * NB)
                # ---- L2: [3072 -> 1536], fp8 DoubleRow
                h2sb = ap_.tile([128, 2 * K3P, NB], dt.float8e4, tag="h2")
                for mt in range(M2):
                    pt = ps.tile([128, NB], dt.float32, tag="ps")
                    for kp in range(K2P):
                        nc.tensor.matmul(
                            pt[:],
                            w2sb[:, 2 * kp:2 * kp + 2, mt * 128:(mt + 1) * 128],
                            h1sb[:, 2 * kp:2 * kp + 2, hs],
                            start=(kp == 0), stop=(kp == K2P - 1),
                            perf_mode=PM.DoubleRow)
                    nc.scalar.activation(h2sb[:, mt, :], pt[:], AF.Sign,
                                         bias=c2sb[:, mt:mt + 1],
                                         scale=a2sb[:, mt:mt + 1])

                # ---- L3: [1536 -> 768], fp8 DoubleRow; output clipped bf16
                # (bf16 keeps L4 single-pass; walrus double-pumps fp32)
                h3c = ap_.tile([128, K4T, NB], dt.bfloat16, tag="h3")
                for mt in range(M3):
                    pt = ps.tile([128, NB], dt.float32, tag="ps")
                    for kp in range(K3P):
                        nc.tensor.matmul(
                            pt[:],
                            w3sb[:, 2 * kp:2 * kp + 2, mt * 128:(mt + 1) * 128],
                            h2sb[:, 2 * kp:2 * kp + 2, :],
                            start=(kp == 0), stop=(kp == K3P - 1),
                            perf_mode=PM.DoubleRow)
                    nc.vector.tensor_scalar(h3c[:, mt, :], pt[:],
                                            a3sb[:, mt:mt + 1],
                                            c3sb[:, mt:mt + 1],
                                            op0=ALU.mult, op1=ALU.add)
                    nc.vector.tensor_scalar(h3c[:, mt, :], h3c[:, mt, :],
                                            1.0, -1.0, op0=ALU.min,
                                            op1=ALU.max)

                # ---- L4: logits z = y3 @ w4.T + b4, [batch-tile, 10]
                for bt in range(NB // 128):
                    gbt = (g * halves + h) * (NB // 128) + bt
                    p4 = ps4.tile([128, NCLS], dt.float32, tag="p4")
                    for kt in range(K4T):
                        nc.tensor.matmul(p4[:],
                                         h3c[:, kt, bt * 128:(bt + 1) * 128],
                                         w4sb[:, kt, :],
                                         start=(kt == 0), stop=(kt == K4T - 1))
                    nc.vector.tensor_add(zout[:, gbt, :], p4[:], b4sb[:])

                if (g == ngroups - 1 and ngroups >= 2 and halves == 2
                        and h == 0):
                    # first half of the last group: epilogue overlaps the
                    # second half's matmuls
                    emit_epilogue(gbts - 4, gbts - 2)

            if g == ngroups - 2:
                # bulk of the log-softmax epilogue hides under the last
                # group's matmuls; only the final row-tiles run in the tail
                emit_epilogue(0, (g + 1) * W // 128)

        if ngroups >= 2 and halves == 2:
            emit_epilogue(gbts - 2, gbts)
        elif ngroups >= 2:
            emit_epilogue((ngroups - 1) * W // 128, gbts)
        else:
            emit_epilogue(0, gbts)

    nc.finalize()
    return nc


def _prep(x, w1, b1, w2, b2, w3, b3, w4, b4,
          g1, be1, m1, v1, g2, be2, m2, v2, g3, be3, m3, v3):
    """Host-side layout prep: transposes, binarized weight casts, BN folds."""
    import concourse.mybir as mybir
    f8 = mybir.dt.np(mybir.dt.float8e4)

    def fold(g, be, m, v, b):
        a = (g / np.sqrt(v + np.float32(BN_EPS))).astype(np.float32)
        c = (a * (b - m) + be).astype(np.float32)
        return a, c

    a1, c1 = fold(g1, be1, m1, v1, b1)
    a2, c2 = fold(g2, be2, m2, v2, b2)
    a3, c3 = fold(g3, be3, m3, v3, b3)

    def cols(v, mtiles):
        return np.ascontiguousarray(v.reshape(mtiles, 128).T)

    pre = dict(
        w1t=np.ascontiguousarray(np.sign(w1).T).astype(np.float16),
        w2t=np.ascontiguousarray(np.sign(w2).T).astype(f8),
        w3t=np.ascontiguousarray(np.sign(w3).T).astype(f8),
        w4t=np.ascontiguousarray(w4.T).astype(mybir.dt.np(mybir.dt.bfloat16)),
        a1s=cols(a1, M1), c1s=cols(c1, M1),
        a2s=cols(a2, M2), c2s=cols(c2, M2),
        a3s=cols(a3, M3), c3s=cols(c3, M3),
        b4s=np.ascontiguousarray(np.tile(b4.astype(np.float32), (128, 1))),
    )
    xt = np.ascontiguousarray(x.T.astype(np.float32))  # [784, B]
    return pre, xt


def run(inputs, **spmd_kwargs):
    from concourse.bass_utils import run_bass_kernel_spmd

    if "nc" not in _cached:
        _cached["nc"] = _build(BC)
    nc = _cached["nc"]

    inputs = {k: np.asarray(v) for k, v in inputs.items()}
    pre, xt = _prep(**inputs)

    in_maps = []
    for core in range(NCORES):
        m = dict(pre)
        m["xt"] = np.ascontiguousarray(xt[:, core * BC:(core + 1) * BC])
        in_maps.append(m)

    res = run_bass_kernel_spmd(nc, in_maps, list(range(NCORES)), **spmd_kwargs)
    outs = [res.results[i]["out"] for i in range(NCORES)]
    return res, np.concatenate(outs, axis=0).astype(np.float32)


def kernel(**inputs):
    return run(inputs)[1]

